# revision 34
# baseline (speedup 1.0000x reference)
"""Trainium2 Bass kernel for BitShiftMamba (2-layer Mamba + LN + scalar head).

v4. Data-parallel over batch (B=8 -> 1 element/core, no collectives).
Channel-major activations (d on partitions, time on free dim).

Key design vs v3:
- selective-scan s-reduction (y = sum_s C_s * h_s) moved to the PE as 16
  accumulating identity matmuls into PSUM (fp32) - removes ~1000 DVE/GpSimd
  tensor_tensor ops per layer
- per-(s,c) tiles are [128, 512]; at = exp(A_s * delta) is ONE ACT op per
  tile (scale = A column); xt = v*B and m = ht*C are single muls split
  between DVE and GpSimd by a static pattern
- y + u*Dp fused into one scalar_tensor_tensor reading y directly from PSUM
- in_proj/conv/x_proj/dt_proj/out_proj all PE matmuls with bf16 weights
- t-half software pipeline: PRE(li+1,tcc) emitted between SCAN phases so
  the PE/ACT pre-chain hides under the DVE scan stream

Self-contained: hardcodes all shapes; imports concourse from /opt/trn_rl_repo.
"""
import sys

if "/opt/trn_rl_repo" not in sys.path:
    sys.path.insert(0, "/opt/trn_rl_repo")

import ml_dtypes
import numpy as np

import concourse.bass as bass
import concourse.tile as tile
from concourse import bacc, mybir
from concourse.bass_utils import run_bass_kernel_spmd
from concourse.tile import add_dep_helper
from concourse import library_config
from concourse import bacc as _bacc_mod
from concourse import hw_specs as _hw_specs

_orig_get_tables = _hw_specs.get_activation_tables


def _patched_tables(arch):
    t = dict(_orig_get_tables(arch))
    ex, ln = mybir.ActivationFunctionType.Exp, mybir.ActivationFunctionType.Ln
    for name in list(t):
        if name != "natural_log_exp_and_others":
            t[name] = t[name] - {ex, ln}
    return t


_bacc_mod.get_activation_tables = _patched_tables

F32 = mybir.dt.float32
BF = mybir.dt.bfloat16
AF = mybir.ActivationFunctionType
OP = mybir.AluOpType
BF_NP = ml_dtypes.bfloat16

# model dims
B, L, DM, DS, DC, NL = 8, 1024, 512, 16, 4, 2
DI = 2 * DM          # 1024
DR = DM // 16        # 32
EPS = 1e-5
NCORES = 8

# kernel tiling
T = 512              # time chunk
NT = L // T          # 2
NDC = DI // 128      # 8 d-chunks
NF = DM // 128       # 4 dm tiles
NFEAT = 2 * DI // 128  # 16 in_proj feature tiles

# engine split: xt goes to GpSimd when (c * DS + s) % XT_GP_MOD == 1
XT_GP_MOD = 4


def _bcast_ap(src: bass.AP, parts: int = 128) -> bass.AP:
    """Partition-broadcast read AP (DRAM source): replicate 1 row to `parts`."""
    return bass.AP(tensor=src.tensor, offset=src.offset,
                   ap=[[0, parts]] + list(src.ap[1:]))


def _build():
    nc = bacc.Bacc("TRN2", target_bir_lowering=False, debug=False,
                   num_devices=NCORES)

    d_rhs2 = nc.dram_tensor("rhs2", [2, L], BF, kind="ExternalInput").ap()
    d_emb = nc.dram_tensor("emb", [2, DM], BF, kind="ExternalInput").ap()
    d_win = nc.dram_tensor("win", [NL, 128, NF, 2 * DI], BF,
                           kind="ExternalInput").ap()
    d_convd = nc.dram_tensor("convd", [NL, 128, NDC, DC, 128], BF,
                             kind="ExternalInput").ap()
    d_wx = nc.dram_tensor("wx", [NL, 128, NDC, DR + 2 * DS], BF,
                          kind="ExternalInput").ap()
    d_wdt = nc.dram_tensor("wdt", [NL, DR, DI], BF, kind="ExternalInput").ap()
    d_wout = nc.dram_tensor("wout", [NL, 128, NDC, DM], BF,
                            kind="ExternalInput").ap()
    d_convb = nc.dram_tensor("convb", [NL, 128, NDC], F32,
                             kind="ExternalInput").ap()
    d_bdt = nc.dram_tensor("bdt", [NL, 128, NDC], F32,
                           kind="ExternalInput").ap()
    d_Dp = nc.dram_tensor("Dp", [NL, 128, NDC], F32, kind="ExternalInput").ap()
    d_A = nc.dram_tensor("A", [NL, 128, NDC, DS], F32,
                         kind="ExternalInput").ap()
    d_ident = nc.dram_tensor("ident", [128, 128], BF,
                             kind="ExternalInput").ap()
    d_gw = nc.dram_tensor("gw", [128, NF], BF, kind="ExternalInput").ap()
    d_gsum = nc.dram_tensor("gsum", [1, 1], F32, kind="ExternalInput").ap()
    d_cbrow = nc.dram_tensor("cbrow", [1, L], F32, kind="ExternalInput").ap()
    d_logits = nc.dram_tensor("logits", [1, L], F32, kind="ExternalOutput").ap()

    with tile.TileContext(nc) as tc:
        with tc.tile_pool(name="wsmall", bufs=1) as wsmall, \
             tc.tile_pool(name="wbig", bufs=1) as wbig, \
             tc.tile_pool(name="state", bufs=1) as state, \
             tc.tile_pool(name="acts", bufs=1) as acts, \
             tc.tile_pool(name="sc", bufs=2) as sc, \
             tc.tile_pool(name="bcp", bufs=1) as bcp, \
             tc.tile_pool(name="ps", bufs=2, space="PSUM") as ps, \
             tc.tile_pool(name="yps", bufs=2, space="PSUM") as yps, \
             tc.tile_pool(name="aps", bufs=4, space="PSUM") as aps, \
             tc.tile_pool(name="dramp", bufs=2, space="DRAM") as dramp:

            # ---- persistent small tensors ----
            rhs2 = state.tile([2, L], BF)
            nc.sync.dma_start(out=rhs2[:], in_=d_rhs2[:])
            embt = state.tile([2, DM], BF)
            nc.sync.dma_start(out=embt[:], in_=d_emb[:])
            ident = state.tile([128, 128], BF)
            nc.sync.dma_start(out=ident[:], in_=d_ident[:])
            gwt = state.tile([128, NF], BF)
            nc.sync.dma_start(out=gwt[:], in_=d_gw[:])
            gsum = state.tile([1, 1], F32)
            nc.sync.dma_start(out=gsum[:], in_=d_gsum[:])
            cbrow = state.tile([1, L], F32)
            nc.sync.dma_start(out=cbrow[:], in_=d_cbrow[:])
            ones_col = state.tile([128, 1], BF)
            nc.vector.memset(ones_col[:], 1.0)
            ones_f32 = state.tile([128, 1], F32)
            nc.vector.memset(ones_f32[:], 1.0)
            # PE p-state warmup: dummy matmuls during weight DMAs
            warm = ps.tile([128, 128], F32, tag="mm", name="warm")
            for wi in range(48):
                nc.tensor.matmul(warm[:], ident[:], ident[:],
                                 start=(wi == 0), stop=(wi == 47))
            eps_col = state.tile([1, 1], F32)
            nc.vector.memset(eps_col[:], EPS)

            wxs, wdts, convbs, bdts, Dps, As = [], [], [], [], [], []
            for li in range(NL):
                wx = wsmall.tile([128, NDC, DR + 2 * DS], BF, tag=f"wx{li}")
                nc.sync.dma_start(out=wx[:], in_=d_wx[li])
                wdt = wsmall.tile([DR, DI], BF, tag=f"wdt{li}")
                nc.sync.dma_start(out=wdt[:], in_=d_wdt[li])
                cb = wsmall.tile([128, NDC], F32, tag=f"cb{li}")
                nc.sync.dma_start(out=cb[:], in_=d_convb[li])
                bd = wsmall.tile([128, NDC], F32, tag=f"bd{li}")
                nc.sync.dma_start(out=bd[:], in_=d_bdt[li])
                dp = wsmall.tile([128, NDC], F32, tag=f"dp{li}")
                nc.sync.dma_start(out=dp[:], in_=d_Dp[li])
                at_ = wsmall.tile([128, NDC, DS], F32, tag=f"A{li}")
                nc.sync.dma_start(out=at_[:], in_=d_A[li])
                wxs.append(wx); wdts.append(wdt); convbs.append(cb)
                bdts.append(bd); Dps.append(dp); As.append(at_)

            wins, convds, wouts = {}, {}, {}

            def load_inconv(li):
                w_in = wbig.tile([128, NF, 2 * DI], BF, tag="win",
                                 name=f"win{li}")
                nc.sync.dma_start(out=w_in[:], in_=d_win[li])
                w_cv = wbig.tile([128, NDC, DC, 128], BF, tag="convd",
                                 name=f"convd{li}")
                nc.sync.dma_start(out=w_cv[:], in_=d_convd[li])
                wins[li] = w_in; convds[li] = w_cv

            def load_wout(li):
                w_out = wbig.tile([128, NDC, DM], BF, tag="wout",
                                  name=f"wout{li}")
                nc.sync.dma_start(out=w_out[:], in_=d_wout[li])
                wouts[li] = w_out

            load_inconv(0)
            load_wout(0)

            def mk(tag_base, cols, dtype, li, n):
                return [[acts.tile([128, cols], dtype, tag=f"{tag_base}{i}_{t}",
                                   name=f"{tag_base}{li}_{i}_{t}")
                         for t in range(NT)] for i in range(n)]

            # h generation 0 (embedding output)
            h_cur = mk("h", T, BF, 0, NF)

            # ---- embedding: h = emb.T @ [1-x; x] ----
            for f in range(NF):
                for t in range(NT):
                    pse = ps.tile([128, T], F32, tag="mm",
                                  name=f"pse{f}_{t}")
                    nc.tensor.matmul(pse[:], embt[:, f * 128:(f + 1) * 128],
                                     rhs2[:, t * T:(t + 1) * T],
                                     start=True, stop=True)
                    nc.scalar.copy(h_cur[f][t][:], pse[:])

            P = {}     # per-layer dict of tiles
            at_prev = {"insts": None}   # at-exp stream of last scanphase

            def pre(li, tcc):
                """in_proj + conv + x_proj + dt_proj + v for (li, tcc)."""
                p = P.setdefault(li, {})
                if tcc == 0:
                    p["up"] = [{} for _ in range(NDC)]
                    p["u"] = mk("u", T, BF, li, NDC)
                    p["zg"] = mk("zg", T, BF, li, NDC)
                    p["delta"] = mk("dl", T, BF, li, NDC)
                    p["v"] = mk("v", T, BF, li, NDC)
                    p["hn"] = mk("h", T, BF, li + 1, NF)
                    p["carry"] = [acts.tile([128, DS], BF, tag=f"carry{c}",
                                            name=f"carry{li}_{c}")
                                  for c in range(NDC)]
                h_in = h_cur if li == 0 else P[li - 1]["hn"]
                silus = []

                for f in range(NDC):
                    upt = acts.tile([128, T + DC - 1], BF, tag=f"up{f}_{tcc}",
                                    name=f"up{li}_{f}_{tcc}")
                    p["up"][f][tcc] = upt
                    if tcc == 0:
                        nc.vector.memset(upt[:, 0:DC - 1], 0.0)
                    else:
                        nc.scalar.copy(upt[:, 0:DC - 1],
                                       p["up"][f][0][:, T:T + DC - 1])

                for f in range(NFEAT):
                    psx = ps.tile([128, T], F32, tag="mm",
                                  name=f"psx{li}_{tcc}_{f}")
                    for kc in range(NF):
                        nc.tensor.matmul(
                            psx[:], wins[li][:, kc, f * 128:(f + 1) * 128],
                            h_in[kc][tcc][:],
                            start=(kc == 0), stop=(kc == NF - 1))
                    if f < NDC:
                        upt = p["up"][f][tcc]
                        nc.vector.tensor_copy(upt[:, DC - 1:DC - 1 + T],
                                              psx[:])
                    else:
                        si = nc.scalar.activation(
                            out=p["zg"][f - NDC][tcc][:], in_=psx[:],
                            func=AF.Silu)
                        if at_prev["insts"] is not None:
                            add_dep_helper(si.ins, at_prev["insts"][96].ins,
                                           sync=False, reason="tbl-defer")
                        silus.append(si)

                for c in range(NDC):
                    psc = ps.tile([128, T], F32, tag="mm",
                                  name=f"psc{li}_{tcc}_{c}")
                    upt = p["up"][c][tcc]
                    for k in range(DC):
                        nc.tensor.matmul(psc[:], convds[li][:, c, k, :],
                                         upt[:, k:k + T],
                                         start=(k == 0), stop=(k == DC - 1))
                    si = nc.scalar.activation(
                        out=p["u"][c][tcc][:], in_=psc[:], func=AF.Silu,
                        bias=convbs[li][:, c:c + 1])
                    if at_prev["insts"] is not None:
                        add_dep_helper(si.ins, at_prev["insts"][96].ins,
                                       sync=False, reason="tbl-defer")
                    silus.append(si)

                xd = ps.tile([DR + 2 * DS, T], F32, tag="mm",
                             name=f"xd{li}_{tcc}")
                for c in range(NDC):
                    nc.tensor.matmul(xd[:], wxs[li][:, c, :],
                                     p["u"][c][tcc][:],
                                     start=(c == 0), stop=(c == NDC - 1))
                dt_sb = sc.tile([DR, T], BF, tag="dtsb")
                nc.vector.tensor_copy(dt_sb[:], xd[0:DR, :])
                bc_sb = sc.tile([2 * DS, T], BF, tag="bcsb")
                nc.vector.tensor_copy(bc_sb[:], xd[DR:DR + 2 * DS, :])
                bcd = dramp.tile([2 * DS, T], BF, tag="bc")
                nc.sync.dma_start(out=bcd[:], in_=bc_sb[:])
                p.setdefault("bcd", {})[tcc] = bcd
                Bb, Cb = [], []
                for s in range(DS):
                    bb = bcp.tile([128, T], BF, tag=f"Bb{s}",
                                  name=f"Bb{li}_{tcc}_{s}")
                    nc.sync.dma_start(out=bb[:],
                                      in_=_bcast_ap(bcd[s:s + 1, :]))
                    cbt = bcp.tile([128, T], BF, tag=f"Cb{s}",
                                   name=f"Cb{li}_{tcc}_{s}")
                    nc.sync.dma_start(
                        out=cbt[:], in_=_bcast_ap(bcd[DS + s:DS + s + 1, :]))
                    Bb.append(bb); Cb.append(cbt)
                p.setdefault("Bb", {})[tcc] = Bb
                p.setdefault("Cb", {})[tcc] = Cb

                for c in range(NDC):
                    psd = ps.tile([128, T], F32, tag="mm",
                                  name=f"psd{li}_{tcc}_{c}")
                    nc.tensor.matmul(psd[:],
                                     wdts[li][:, c * 128:(c + 1) * 128],
                                     dt_sb[:], start=True, stop=True)
                    ed = sc.tile([128, T], BF, tag="ed", bufs=2)
                    e1 = nc.scalar.activation(
                        out=ed[:], in_=psd[:], func=AF.Exp,
                        bias=bdts[li][:, c:c + 1])
                    add_dep_helper(e1.ins, silus[-1].ins, sync=False,
                                   reason="table-cluster")
                    nc.scalar.activation(
                        out=p["delta"][c][tcc][:], in_=ed[:], func=AF.Ln,
                        bias=1.0)
                    nc.gpsimd.tensor_mul(p["v"][c][tcc][:],
                                          p["delta"][c][tcc][:],
                                          p["u"][c][tcc][:])

            def scanphase(li, tcc):
                p = P[li]
                Bb, Cb = p["Bb"][tcc], p["Cb"][tcc]

                ats = []
                for cp in range(NDC // 2):
                    c0, c1 = 2 * cp, 2 * cp + 1
                    yp0 = yps.tile([128, T], F32, tag="y", bufs=2,
                                   name=f"yp{li}_{tcc}_{c0}")
                    yp1 = yps.tile([128, T], F32, tag="y", bufs=2,
                                   name=f"yp{li}_{tcc}_{c1}")
                    pend = {}

                    def emit_gen(si_):
                        pend[si_] = []
                        for c, yp, s in ((c0, yp0, si_),
                                         (c1, yp1, (si_ + 8) % DS)):
                            at = aps.tile([128, T], F32, tag="at")
                            ai = nc.scalar.activation(
                                out=at[:], in_=p["delta"][c][tcc][:],
                                func=AF.Exp, scale=As[li][:, c, s:s + 1])
                            ats.append(ai)
                            xt = sc.tile([128, T], BF, tag="xt", bufs=8)
                            xeng = nc.vector if c == c0 else nc.gpsimd
                            xeng.tensor_mul(xt[:], p["v"][c][tcc][:],
                                            Bb[s][:])
                            pend[si_].append((c, yp, at, xt, s))

                    emit_gen(0)
                    emit_gen(1)
                    for si_ in range(DS):
                        if si_ + 2 < DS:
                            emit_gen(si_ + 2)
                        for c, yp, at, xt, s in pend.pop(si_):
                            ht = sc.tile([128, T], BF, tag="ht", bufs=6)
                            init = (0.0 if tcc == 0
                                    else p["carry"][c][:, s:s + 1])
                            nc.vector.tensor_tensor_scan(
                                out=ht[:], data0=at[:], data1=xt[:],
                                initial=init, op0=OP.mult, op1=OP.add)
                            if tcc < NT - 1:
                                nc.scalar.copy(p["carry"][c][:, s:s + 1],
                                               ht[:, T - 1:T])
                            m = sc.tile([128, T], BF, tag="m", bufs=6)
                            meng = (nc.vector if (c == c0 and s % 4 == 0)
                                    else nc.gpsimd)
                            meng.tensor_mul(m[:], ht[:], Cb[s][:])
                            nc.tensor.matmul(yp[:], ident[:], m[:],
                                             start=(si_ == 0),
                                             stop=(si_ == DS - 1))
                    for c, yp in ((c0, yp0), (c1, yp1)):
                        yf = sc.tile([128, T], BF, tag="yf", bufs=2)
                        nc.vector.scalar_tensor_tensor(
                            out=yf[:], in0=p["u"][c][tcc][:],
                            scalar=Dps[li][:, c:c + 1], in1=yp[:],
                            op0=OP.mult, op1=OP.add)
                        ygt = acts.tile([128, T + DC - 1], BF,
                                        tag=f"up{c}_{tcc}",
                                        name=f"yg{li}_{c}_{tcc}")
                        p.setdefault("yg", {}).setdefault(c, {})[tcc] = ygt
                        nc.gpsimd.tensor_mul(ygt[:, 0:T], yf[:],
                                              p["zg"][c][tcc][:])

                # out_proj for this half
                for f in range(NF):
                    po = ps.tile([128, T], F32, tag="mm",
                                 name=f"po{li}_{tcc}_{f}")
                    for c in range(NDC):
                        nc.tensor.matmul(
                            po[:], wouts[li][:, c, f * 128:(f + 1) * 128],
                            p["yg"][c][tcc][:, 0:T],
                            start=(c == 0), stop=(c == NDC - 1))
                    nc.vector.tensor_copy(p["hn"][f][tcc][:], po[:])
                at_prev["insts"] = ats

            # ---- software-pipelined emission ----
            pre(0, 0)
            pre(0, 1)
            load_inconv(1)
            scanphase(0, 0)
            pre(1, 0)
            scanphase(0, 1)
            load_wout(1)
            pre(1, 1)
            scanphase(1, 0)

            # ---- final layernorm + head (emitted per t-half) ----
            def ln_head(t):
                h_fin = P[NL - 1]["hn"]
                sl = slice(t * T, (t + 1) * T)
                psl = ps.tile([128, T], F32, tag="mm", name=f"psl{t}")
                s0, s1, s2 = psl[0:1, :], psl[32:33, :], psl[64:65, :]
                for f in range(NF):
                    nc.tensor.matmul(s0, ones_col[:], h_fin[f][t][:],
                                     start=(f == 0), stop=(f == NF - 1),
                                     skip_group_check=True)
                    nc.tensor.matmul(s1, gwt[:, f:f + 1], h_fin[f][t][:],
                                     start=(f == 0), stop=(f == NF - 1),
                                     skip_group_check=True)
                    sq = sc.tile([128, T], BF, tag="sq", bufs=2)
                    nc.scalar.activation(out=sq[:], in_=h_fin[f][t][:],
                                         func=AF.Square)
                    nc.tensor.matmul(s2, ones_col[:], sq[:],
                                     start=(f == 0), stop=(f == NF - 1),
                                     skip_group_check=True)
                mu = sc.tile([1, T], F32, tag="hd", bufs=4)
                nc.vector.tensor_scalar_mul(mu[:], s0, 1.0 / DM)
                musq = sc.tile([1, T], F32, tag="hd", bufs=4)
                nc.vector.tensor_mul(musq[:], mu[:], mu[:])
                var = sc.tile([1, T], F32, tag="hd", bufs=4)
                nc.vector.scalar_tensor_tensor(out=var[:], in0=s2,
                                               scalar=1.0 / DM, in1=musq[:],
                                               op0=OP.mult, op1=OP.subtract)
                negnum = sc.tile([1, T], F32, tag="hd", bufs=4)
                nc.vector.scalar_tensor_tensor(out=negnum[:], in0=mu[:],
                                               scalar=gsum[:, 0:1], in1=s1,
                                               op0=OP.mult, op1=OP.subtract)
                lnv = sc.tile([1, T], F32, tag="hd", bufs=4)
                nc.scalar.activation(out=lnv[:], in_=var[:], func=AF.Ln,
                                     bias=eps_col[:, 0:1])
                rstd = sc.tile([1, T], F32, tag="hd", bufs=4)
                nc.scalar.activation(out=rstd[:], in_=lnv[:], func=AF.Exp,
                                     scale=-0.5)
                t1 = sc.tile([1, T], F32, tag="hd", bufs=4)
                nc.vector.tensor_mul(t1[:], negnum[:], rstd[:])
                lg = sc.tile([1, T], F32, tag="hd", bufs=4)
                nc.vector.scalar_tensor_tensor(out=lg[:], in0=t1[:],
                                               scalar=-1.0, in1=cbrow[:, sl],
                                               op0=OP.mult, op1=OP.add)
                nc.sync.dma_start(out=d_logits[:, sl], in_=lg[:])

            ln_head(0)
            scanphase(1, 1)
            ln_head(1)

    nc.compile()
    return nc


_NC = None
_last_in_maps = None


def kernel(**inputs) -> np.ndarray:
    global _NC, _last_in_maps
    if _NC is None:
        _NC = _build()
    nc = _NC

    x = np.asarray(inputs["x"])
    emb = np.asarray(inputs["emb"], np.float32)
    Win = np.asarray(inputs["Win"], np.float32)
    conv_w = np.asarray(inputs["conv_w"], np.float32)
    conv_b = np.asarray(inputs["conv_b"], np.float32)
    Wx = np.asarray(inputs["Wx"], np.float32)
    Wdt = np.asarray(inputs["Wdt"], np.float32)
    bdt = np.asarray(inputs["bdt"], np.float32)
    A_log = np.asarray(inputs["A_log"], np.float32)
    Dp = np.asarray(inputs["Dp"], np.float32)
    Wout = np.asarray(inputs["Wout"], np.float32)
    ln_g = np.asarray(inputs["ln_g"], np.float32)
    ln_b = np.asarray(inputs["ln_b"], np.float32)
    head_w = np.asarray(inputs["head_w"], np.float32)
    head_b = np.asarray(inputs["head_b"], np.float32)

    def bf(a):
        return np.ascontiguousarray(a).astype(BF_NP)

    winT = np.swapaxes(Win, 1, 2)                       # (NL, DM, 2DI)
    win = bf(winT.reshape(NL, NF, 128, 2 * DI).transpose(0, 2, 1, 3))
    convd = np.zeros((NL, 128, NDC, DC, 128), np.float32)
    idx = np.arange(128)
    for li in range(NL):
        for c in range(NDC):
            for k in range(DC):
                convd[li, idx, c, k, idx] = conv_w[li, c * 128:(c + 1) * 128, k]
    wxT = np.swapaxes(Wx, 1, 2)                         # (NL, DI, DR+2DS)
    wx = bf(wxT.reshape(NL, NDC, 128, DR + 2 * DS).transpose(0, 2, 1, 3))
    wdt = bf(np.swapaxes(Wdt, 1, 2))                    # (NL, DR, DI)
    woutT = np.swapaxes(Wout, 1, 2)                     # (NL, DI, DM)
    wout = bf(woutT.reshape(NL, NDC, 128, DM).transpose(0, 2, 1, 3))
    A = -np.exp(A_log)                                  # (NL, DI, DS)
    Af = np.ascontiguousarray(
        A.reshape(NL, NDC, 128, DS).transpose(0, 2, 1, 3), dtype=np.float32)

    def pcol(a):   # (NL, DI) -> (NL, 128, NDC), di = c*128 + p
        return np.ascontiguousarray(
            a.reshape(NL, NDC, 128).transpose(0, 2, 1), dtype=np.float32)

    gw = (ln_g * head_w).astype(np.float32)
    gwt = bf(gw.reshape(NF, 128).T)
    gsum = np.full((1, 1), gw.sum(), np.float32)
    cb = float((ln_b * head_w).sum() + head_b)
    cbrow = np.full((1, L), cb, np.float32)

    shared = {
        "emb": bf(emb), "win": win, "convd": bf(convd), "wx": wx,
        "wdt": wdt, "wout": wout, "convb": pcol(conv_b), "bdt": pcol(bdt),
        "Dp": pcol(Dp), "A": Af, "gw": gwt, "gsum": gsum, "cbrow": cbrow,
        "ident": bf(np.eye(128, dtype=np.float32)),
    }
    in_maps = []
    for bi in range(NCORES):
        xb = x[bi].astype(np.float32)
        rhs2 = np.stack([1.0 - xb, xb]).astype(BF_NP)   # (2, L)
        m = dict(shared)
        m["rhs2"] = rhs2
        in_maps.append(m)

    _last_in_maps = in_maps
    res = run_bass_kernel_spmd(nc, in_maps, list(range(NCORES)))
    out = np.stack([res.results[i]["logits"][0] for i in range(NCORES)])
    return out.astype(np.float32)


# revision 35
# speedup vs baseline: 1.0044x; 1.0044x over previous
"""Trainium2 Bass kernel for BitShiftMamba (2-layer Mamba + LN + scalar head).

v4. Data-parallel over batch (B=8 -> 1 element/core, no collectives).
Channel-major activations (d on partitions, time on free dim).

Key design vs v3:
- selective-scan s-reduction (y = sum_s C_s * h_s) moved to the PE as 16
  accumulating identity matmuls into PSUM (fp32) - removes ~1000 DVE/GpSimd
  tensor_tensor ops per layer
- per-(s,c) tiles are [128, 512]; at = exp(A_s * delta) is ONE ACT op per
  tile (scale = A column); xt = v*B and m = ht*C are single muls split
  between DVE and GpSimd by a static pattern
- y + u*Dp fused into one scalar_tensor_tensor reading y directly from PSUM
- in_proj/conv/x_proj/dt_proj/out_proj all PE matmuls with bf16 weights
- t-half software pipeline: PRE(li+1,tcc) emitted between SCAN phases so
  the PE/ACT pre-chain hides under the DVE scan stream

Self-contained: hardcodes all shapes; imports concourse from /opt/trn_rl_repo.
"""
import sys

if "/opt/trn_rl_repo" not in sys.path:
    sys.path.insert(0, "/opt/trn_rl_repo")

import ml_dtypes
import numpy as np

import concourse.bass as bass
import concourse.tile as tile
from concourse import bacc, mybir
from concourse.bass_utils import run_bass_kernel_spmd
from concourse.tile import add_dep_helper
from concourse import library_config
from concourse import bacc as _bacc_mod
from concourse import hw_specs as _hw_specs

_orig_get_tables = _hw_specs.get_activation_tables


def _patched_tables(arch):
    t = dict(_orig_get_tables(arch))
    ex, ln = mybir.ActivationFunctionType.Exp, mybir.ActivationFunctionType.Ln
    for name in list(t):
        if name != "natural_log_exp_and_others":
            t[name] = t[name] - {ex, ln}
    return t


_bacc_mod.get_activation_tables = _patched_tables

F32 = mybir.dt.float32
BF = mybir.dt.bfloat16
AF = mybir.ActivationFunctionType
OP = mybir.AluOpType
BF_NP = ml_dtypes.bfloat16

# model dims
B, L, DM, DS, DC, NL = 8, 1024, 512, 16, 4, 2
DI = 2 * DM          # 1024
DR = DM // 16        # 32
EPS = 1e-5
NCORES = 8

# kernel tiling
T = 512              # time chunk
NT = L // T          # 2
NDC = DI // 128      # 8 d-chunks
NF = DM // 128       # 4 dm tiles
NFEAT = 2 * DI // 128  # 16 in_proj feature tiles

# engine split: xt goes to GpSimd when (c * DS + s) % XT_GP_MOD == 1
XT_GP_MOD = 4


def _bcast_ap(src: bass.AP, parts: int = 128) -> bass.AP:
    """Partition-broadcast read AP (DRAM source): replicate 1 row to `parts`."""
    return bass.AP(tensor=src.tensor, offset=src.offset,
                   ap=[[0, parts]] + list(src.ap[1:]))


def _build():
    nc = bacc.Bacc("TRN2", target_bir_lowering=False, debug=False,
                   num_devices=NCORES)

    d_rhs2 = nc.dram_tensor("rhs2", [2, L], BF, kind="ExternalInput").ap()
    d_emb = nc.dram_tensor("emb", [2, DM], BF, kind="ExternalInput").ap()
    d_win = nc.dram_tensor("win", [NL, 128, NF, 2 * DI], BF,
                           kind="ExternalInput").ap()
    d_convd = nc.dram_tensor("convd", [NL, 128, NDC, DC, 128], BF,
                             kind="ExternalInput").ap()
    d_wx = nc.dram_tensor("wx", [NL, 128, NDC, DR + 2 * DS], BF,
                          kind="ExternalInput").ap()
    d_wdt = nc.dram_tensor("wdt", [NL, DR, DI], BF, kind="ExternalInput").ap()
    d_wout = nc.dram_tensor("wout", [NL, 128, NDC, DM], BF,
                            kind="ExternalInput").ap()
    d_convb = nc.dram_tensor("convb", [NL, 128, NDC], F32,
                             kind="ExternalInput").ap()
    d_bdt = nc.dram_tensor("bdt", [NL, 128, NDC], F32,
                           kind="ExternalInput").ap()
    d_Dp = nc.dram_tensor("Dp", [NL, 128, NDC], F32, kind="ExternalInput").ap()
    d_A = nc.dram_tensor("A", [NL, 128, NDC, DS], F32,
                         kind="ExternalInput").ap()
    d_ident = nc.dram_tensor("ident", [128, 128], BF,
                             kind="ExternalInput").ap()
    d_gw = nc.dram_tensor("gw", [128, NF], BF, kind="ExternalInput").ap()
    d_gsum = nc.dram_tensor("gsum", [1, 1], F32, kind="ExternalInput").ap()
    d_cbrow = nc.dram_tensor("cbrow", [1, L], F32, kind="ExternalInput").ap()
    d_logits = nc.dram_tensor("logits", [1, L], F32, kind="ExternalOutput").ap()

    with tile.TileContext(nc) as tc:
        with tc.tile_pool(name="wsmall", bufs=1) as wsmall, \
             tc.tile_pool(name="wbig", bufs=1) as wbig, \
             tc.tile_pool(name="state", bufs=1) as state, \
             tc.tile_pool(name="acts", bufs=1) as acts, \
             tc.tile_pool(name="sc", bufs=2) as sc, \
             tc.tile_pool(name="bcp", bufs=1) as bcp, \
             tc.tile_pool(name="ps", bufs=2, space="PSUM") as ps, \
             tc.tile_pool(name="yps", bufs=2, space="PSUM") as yps, \
             tc.tile_pool(name="aps", bufs=4, space="PSUM") as aps, \
             tc.tile_pool(name="dramp", bufs=2, space="DRAM") as dramp:

            # ---- persistent small tensors ----
            rhs2 = state.tile([2, L], BF)
            nc.sync.dma_start(out=rhs2[:], in_=d_rhs2[:])
            embt = state.tile([2, DM], BF)
            nc.sync.dma_start(out=embt[:], in_=d_emb[:])
            ident = state.tile([128, 128], BF)
            nc.sync.dma_start(out=ident[:], in_=d_ident[:])
            gwt = state.tile([128, NF], BF)
            nc.sync.dma_start(out=gwt[:], in_=d_gw[:])
            gsum = state.tile([1, 1], F32)
            nc.sync.dma_start(out=gsum[:], in_=d_gsum[:])
            cbrow = state.tile([1, L], F32)
            nc.sync.dma_start(out=cbrow[:], in_=d_cbrow[:])
            ones_col = state.tile([128, 1], BF)
            nc.vector.memset(ones_col[:], 1.0)
            ones_f32 = state.tile([128, 1], F32)
            nc.vector.memset(ones_f32[:], 1.0)
            # PE p-state warmup: dummy matmuls during weight DMAs
            warm = ps.tile([128, 128], F32, tag="mm", name="warm")
            for wi in range(48):
                nc.tensor.matmul(warm[:], ident[:], ident[:],
                                 start=(wi == 0), stop=(wi == 47))
            eps_col = state.tile([1, 1], F32)
            nc.vector.memset(eps_col[:], EPS)

            wxs, wdts, convbs, bdts, Dps, As = [], [], [], [], [], []
            for li in range(NL):
                wx = wsmall.tile([128, NDC, DR + 2 * DS], BF, tag=f"wx{li}")
                nc.sync.dma_start(out=wx[:], in_=d_wx[li])
                wdt = wsmall.tile([DR, DI], BF, tag=f"wdt{li}")
                nc.sync.dma_start(out=wdt[:], in_=d_wdt[li])
                cb = wsmall.tile([128, NDC], F32, tag=f"cb{li}")
                nc.sync.dma_start(out=cb[:], in_=d_convb[li])
                bd = wsmall.tile([128, NDC], F32, tag=f"bd{li}")
                nc.sync.dma_start(out=bd[:], in_=d_bdt[li])
                dp = wsmall.tile([128, NDC], F32, tag=f"dp{li}")
                nc.sync.dma_start(out=dp[:], in_=d_Dp[li])
                at_ = wsmall.tile([128, NDC, DS], F32, tag=f"A{li}")
                nc.sync.dma_start(out=at_[:], in_=d_A[li])
                wxs.append(wx); wdts.append(wdt); convbs.append(cb)
                bdts.append(bd); Dps.append(dp); As.append(at_)

            wins, convds, wouts = {}, {}, {}

            def load_inconv(li):
                w_in = wbig.tile([128, NF, 2 * DI], BF, tag="win",
                                 name=f"win{li}")
                nc.sync.dma_start(out=w_in[:], in_=d_win[li])
                w_cv = wbig.tile([128, NDC, DC, 128], BF, tag="convd",
                                 name=f"convd{li}")
                nc.sync.dma_start(out=w_cv[:], in_=d_convd[li])
                wins[li] = w_in; convds[li] = w_cv

            def load_wout(li):
                w_out = wbig.tile([128, NDC, DM], BF, tag="wout",
                                  name=f"wout{li}")
                nc.sync.dma_start(out=w_out[:], in_=d_wout[li])
                wouts[li] = w_out

            load_inconv(0)
            load_wout(0)

            def mk(tag_base, cols, dtype, li, n):
                return [[acts.tile([128, cols], dtype, tag=f"{tag_base}{i}_{t}",
                                   name=f"{tag_base}{li}_{i}_{t}")
                         for t in range(NT)] for i in range(n)]

            # h generation 0 (embedding output)
            h_cur = mk("h", T, BF, 0, NF)

            # ---- embedding: h = emb.T @ [1-x; x] ----
            for f in range(NF):
                for t in range(NT):
                    pse = ps.tile([128, T], F32, tag="mm",
                                  name=f"pse{f}_{t}")
                    nc.tensor.matmul(pse[:], embt[:, f * 128:(f + 1) * 128],
                                     rhs2[:, t * T:(t + 1) * T],
                                     start=True, stop=True)
                    nc.scalar.copy(h_cur[f][t][:], pse[:])

            P = {}     # per-layer dict of tiles
            at_prev = {"insts": None}   # at-exp stream of last scanphase

            def pre(li, tcc):
                """in_proj + conv + x_proj + dt_proj + v for (li, tcc)."""
                p = P.setdefault(li, {})
                if tcc == 0:
                    p["up"] = [{} for _ in range(NDC)]
                    p["u"] = mk("u", T, BF, li, NDC)
                    p["zg"] = mk("zg", T, BF, li, NDC)
                    p["delta"] = mk("dl", T, BF, li, NDC)
                    p["v"] = mk("v", T, BF, li, NDC)
                    p["hn"] = mk("h", T, BF, li + 1, NF)
                    p["carry"] = [acts.tile([128, DS], BF, tag=f"carry{c}",
                                            name=f"carry{li}_{c}")
                                  for c in range(NDC)]
                h_in = h_cur if li == 0 else P[li - 1]["hn"]
                silus = []

                for f in range(NDC):
                    upt = acts.tile([128, T + DC - 1], BF, tag=f"up{f}_{tcc}",
                                    name=f"up{li}_{f}_{tcc}")
                    p["up"][f][tcc] = upt
                    if tcc == 0:
                        nc.vector.memset(upt[:, 0:DC - 1], 0.0)
                    else:
                        nc.scalar.copy(upt[:, 0:DC - 1],
                                       p["up"][f][0][:, T:T + DC - 1])

                for f in range(NFEAT):
                    psx = ps.tile([128, T], F32, tag="mm",
                                  name=f"psx{li}_{tcc}_{f}")
                    for kc in range(NF):
                        nc.tensor.matmul(
                            psx[:], wins[li][:, kc, f * 128:(f + 1) * 128],
                            h_in[kc][tcc][:],
                            start=(kc == 0), stop=(kc == NF - 1))
                    if f < NDC:
                        upt = p["up"][f][tcc]
                        nc.scalar.copy(upt[:, DC - 1:DC - 1 + T], psx[:])
                    else:
                        si = nc.scalar.activation(
                            out=p["zg"][f - NDC][tcc][:], in_=psx[:],
                            func=AF.Silu)
                        if at_prev["insts"] is not None:
                            add_dep_helper(si.ins, at_prev["insts"][96].ins,
                                           sync=False, reason="tbl-defer")
                        silus.append(si)

                for c in range(NDC):
                    psc = ps.tile([128, T], F32, tag="mm",
                                  name=f"psc{li}_{tcc}_{c}")
                    upt = p["up"][c][tcc]
                    for k in range(DC):
                        nc.tensor.matmul(psc[:], convds[li][:, c, k, :],
                                         upt[:, k:k + T],
                                         start=(k == 0), stop=(k == DC - 1))
                    si = nc.scalar.activation(
                        out=p["u"][c][tcc][:], in_=psc[:], func=AF.Silu,
                        bias=convbs[li][:, c:c + 1])
                    if at_prev["insts"] is not None:
                        add_dep_helper(si.ins, at_prev["insts"][96].ins,
                                       sync=False, reason="tbl-defer")
                    silus.append(si)

                xd = ps.tile([DR + 2 * DS, T], F32, tag="mm",
                             name=f"xd{li}_{tcc}")
                for c in range(NDC):
                    nc.tensor.matmul(xd[:], wxs[li][:, c, :],
                                     p["u"][c][tcc][:],
                                     start=(c == 0), stop=(c == NDC - 1))
                dt_sb = sc.tile([DR, T], BF, tag="dtsb")
                nc.vector.tensor_copy(dt_sb[:], xd[0:DR, :])
                bc_sb = sc.tile([2 * DS, T], BF, tag="bcsb")
                nc.vector.tensor_copy(bc_sb[:], xd[DR:DR + 2 * DS, :])
                bcd = dramp.tile([2 * DS, T], BF, tag="bc")
                nc.sync.dma_start(out=bcd[:], in_=bc_sb[:])
                p.setdefault("bcd", {})[tcc] = bcd
                Bb, Cb = [], []
                for s in range(DS):
                    bb = bcp.tile([128, T], BF, tag=f"Bb{s}",
                                  name=f"Bb{li}_{tcc}_{s}")
                    nc.sync.dma_start(out=bb[:],
                                      in_=_bcast_ap(bcd[s:s + 1, :]))
                    cbt = bcp.tile([128, T], BF, tag=f"Cb{s}",
                                   name=f"Cb{li}_{tcc}_{s}")
                    nc.sync.dma_start(
                        out=cbt[:], in_=_bcast_ap(bcd[DS + s:DS + s + 1, :]))
                    Bb.append(bb); Cb.append(cbt)
                p.setdefault("Bb", {})[tcc] = Bb
                p.setdefault("Cb", {})[tcc] = Cb

                for c in range(NDC):
                    psd = ps.tile([128, T], F32, tag="mm",
                                  name=f"psd{li}_{tcc}_{c}")
                    nc.tensor.matmul(psd[:],
                                     wdts[li][:, c * 128:(c + 1) * 128],
                                     dt_sb[:], start=True, stop=True)
                    ed = sc.tile([128, T], BF, tag="ed", bufs=2)
                    e1 = nc.scalar.activation(
                        out=ed[:], in_=psd[:], func=AF.Exp,
                        bias=bdts[li][:, c:c + 1])
                    add_dep_helper(e1.ins, silus[-1].ins, sync=False,
                                   reason="table-cluster")
                    nc.scalar.activation(
                        out=p["delta"][c][tcc][:], in_=ed[:], func=AF.Ln,
                        bias=1.0)
                    nc.gpsimd.tensor_mul(p["v"][c][tcc][:],
                                          p["delta"][c][tcc][:],
                                          p["u"][c][tcc][:])

            def scanphase(li, tcc):
                p = P[li]
                Bb, Cb = p["Bb"][tcc], p["Cb"][tcc]

                ats = []
                for cp in range(NDC // 2):
                    c0, c1 = 2 * cp, 2 * cp + 1
                    yp0 = yps.tile([128, T], F32, tag="y", bufs=2,
                                   name=f"yp{li}_{tcc}_{c0}")
                    yp1 = yps.tile([128, T], F32, tag="y", bufs=2,
                                   name=f"yp{li}_{tcc}_{c1}")
                    pend = {}

                    def emit_gen(si_):
                        pend[si_] = []
                        for c, yp, s in ((c0, yp0, si_),
                                         (c1, yp1, (si_ + 8) % DS)):
                            at = aps.tile([128, T], F32, tag="at")
                            ai = nc.scalar.activation(
                                out=at[:], in_=p["delta"][c][tcc][:],
                                func=AF.Exp, scale=As[li][:, c, s:s + 1])
                            ats.append(ai)
                            xt = sc.tile([128, T], BF, tag="xt", bufs=8)
                            xeng = nc.vector if c == c0 else nc.gpsimd
                            xeng.tensor_mul(xt[:], p["v"][c][tcc][:],
                                            Bb[s][:])
                            pend[si_].append((c, yp, at, xt, s))

                    emit_gen(0)
                    emit_gen(1)
                    for si_ in range(DS):
                        if si_ + 2 < DS:
                            emit_gen(si_ + 2)
                        for c, yp, at, xt, s in pend.pop(si_):
                            ht = sc.tile([128, T], BF, tag="ht", bufs=6)
                            init = (0.0 if tcc == 0
                                    else p["carry"][c][:, s:s + 1])
                            nc.vector.tensor_tensor_scan(
                                out=ht[:], data0=at[:], data1=xt[:],
                                initial=init, op0=OP.mult, op1=OP.add)
                            if tcc < NT - 1:
                                nc.scalar.copy(p["carry"][c][:, s:s + 1],
                                               ht[:, T - 1:T])
                            m = sc.tile([128, T], BF, tag="m", bufs=6)
                            meng = (nc.vector if (c == c0 and s % 4 == 0)
                                    else nc.gpsimd)
                            meng.tensor_mul(m[:], ht[:], Cb[s][:])
                            nc.tensor.matmul(yp[:], ident[:], m[:],
                                             start=(si_ == 0),
                                             stop=(si_ == DS - 1))
                    for c, yp in ((c0, yp0), (c1, yp1)):
                        yf = sc.tile([128, T], BF, tag="yf", bufs=2)
                        nc.vector.scalar_tensor_tensor(
                            out=yf[:], in0=p["u"][c][tcc][:],
                            scalar=Dps[li][:, c:c + 1], in1=yp[:],
                            op0=OP.mult, op1=OP.add)
                        ygt = acts.tile([128, T + DC - 1], BF,
                                        tag=f"up{c}_{tcc}",
                                        name=f"yg{li}_{c}_{tcc}")
                        p.setdefault("yg", {}).setdefault(c, {})[tcc] = ygt
                        nc.gpsimd.tensor_mul(ygt[:, 0:T], yf[:],
                                              p["zg"][c][tcc][:])

                # out_proj for this half
                for f in range(NF):
                    po = ps.tile([128, T], F32, tag="mm",
                                 name=f"po{li}_{tcc}_{f}")
                    for c in range(NDC):
                        nc.tensor.matmul(
                            po[:], wouts[li][:, c, f * 128:(f + 1) * 128],
                            p["yg"][c][tcc][:, 0:T],
                            start=(c == 0), stop=(c == NDC - 1))
                    nc.scalar.copy(p["hn"][f][tcc][:], po[:])
                at_prev["insts"] = ats

            # ---- software-pipelined emission ----
            pre(0, 0)
            pre(0, 1)
            load_inconv(1)
            scanphase(0, 0)
            pre(1, 0)
            scanphase(0, 1)
            load_wout(1)
            pre(1, 1)
            scanphase(1, 0)

            # ---- final layernorm + head (emitted per t-half) ----
            def ln_head(t):
                h_fin = P[NL - 1]["hn"]
                sl = slice(t * T, (t + 1) * T)
                psl = ps.tile([128, T], F32, tag="mm", name=f"psl{t}")
                s0, s1, s2 = psl[0:1, :], psl[32:33, :], psl[64:65, :]
                for f in range(NF):
                    nc.tensor.matmul(s0, ones_col[:], h_fin[f][t][:],
                                     start=(f == 0), stop=(f == NF - 1),
                                     skip_group_check=True)
                    nc.tensor.matmul(s1, gwt[:, f:f + 1], h_fin[f][t][:],
                                     start=(f == 0), stop=(f == NF - 1),
                                     skip_group_check=True)
                    sq = sc.tile([128, T], BF, tag="sq", bufs=2)
                    nc.scalar.activation(out=sq[:], in_=h_fin[f][t][:],
                                         func=AF.Square)
                    nc.tensor.matmul(s2, ones_col[:], sq[:],
                                     start=(f == 0), stop=(f == NF - 1),
                                     skip_group_check=True)
                mu = sc.tile([1, T], F32, tag="hd", bufs=4)
                nc.vector.tensor_scalar_mul(mu[:], s0, 1.0 / DM)
                musq = sc.tile([1, T], F32, tag="hd", bufs=4)
                nc.vector.tensor_mul(musq[:], mu[:], mu[:])
                var = sc.tile([1, T], F32, tag="hd", bufs=4)
                nc.vector.scalar_tensor_tensor(out=var[:], in0=s2,
                                               scalar=1.0 / DM, in1=musq[:],
                                               op0=OP.mult, op1=OP.subtract)
                negnum = sc.tile([1, T], F32, tag="hd", bufs=4)
                nc.vector.scalar_tensor_tensor(out=negnum[:], in0=mu[:],
                                               scalar=gsum[:, 0:1], in1=s1,
                                               op0=OP.mult, op1=OP.subtract)
                lnv = sc.tile([1, T], F32, tag="hd", bufs=4)
                nc.scalar.activation(out=lnv[:], in_=var[:], func=AF.Ln,
                                     bias=eps_col[:, 0:1])
                rstd = sc.tile([1, T], F32, tag="hd", bufs=4)
                nc.scalar.activation(out=rstd[:], in_=lnv[:], func=AF.Exp,
                                     scale=-0.5)
                t1 = sc.tile([1, T], F32, tag="hd", bufs=4)
                nc.vector.tensor_mul(t1[:], negnum[:], rstd[:])
                lg = sc.tile([1, T], F32, tag="hd", bufs=4)
                nc.vector.scalar_tensor_tensor(out=lg[:], in0=t1[:],
                                               scalar=-1.0, in1=cbrow[:, sl],
                                               op0=OP.mult, op1=OP.add)
                nc.sync.dma_start(out=d_logits[:, sl], in_=lg[:])

            ln_head(0)
            scanphase(1, 1)
            ln_head(1)

    nc.compile()
    return nc


_NC = None
_last_in_maps = None


def kernel(**inputs) -> np.ndarray:
    global _NC, _last_in_maps
    if _NC is None:
        _NC = _build()
    nc = _NC

    x = np.asarray(inputs["x"])
    emb = np.asarray(inputs["emb"], np.float32)
    Win = np.asarray(inputs["Win"], np.float32)
    conv_w = np.asarray(inputs["conv_w"], np.float32)
    conv_b = np.asarray(inputs["conv_b"], np.float32)
    Wx = np.asarray(inputs["Wx"], np.float32)
    Wdt = np.asarray(inputs["Wdt"], np.float32)
    bdt = np.asarray(inputs["bdt"], np.float32)
    A_log = np.asarray(inputs["A_log"], np.float32)
    Dp = np.asarray(inputs["Dp"], np.float32)
    Wout = np.asarray(inputs["Wout"], np.float32)
    ln_g = np.asarray(inputs["ln_g"], np.float32)
    ln_b = np.asarray(inputs["ln_b"], np.float32)
    head_w = np.asarray(inputs["head_w"], np.float32)
    head_b = np.asarray(inputs["head_b"], np.float32)

    def bf(a):
        return np.ascontiguousarray(a).astype(BF_NP)

    winT = np.swapaxes(Win, 1, 2)                       # (NL, DM, 2DI)
    win = bf(winT.reshape(NL, NF, 128, 2 * DI).transpose(0, 2, 1, 3))
    convd = np.zeros((NL, 128, NDC, DC, 128), np.float32)
    idx = np.arange(128)
    for li in range(NL):
        for c in range(NDC):
            for k in range(DC):
                convd[li, idx, c, k, idx] = conv_w[li, c * 128:(c + 1) * 128, k]
    wxT = np.swapaxes(Wx, 1, 2)                         # (NL, DI, DR+2DS)
    wx = bf(wxT.reshape(NL, NDC, 128, DR + 2 * DS).transpose(0, 2, 1, 3))
    wdt = bf(np.swapaxes(Wdt, 1, 2))                    # (NL, DR, DI)
    woutT = np.swapaxes(Wout, 1, 2)                     # (NL, DI, DM)
    wout = bf(woutT.reshape(NL, NDC, 128, DM).transpose(0, 2, 1, 3))
    A = -np.exp(A_log)                                  # (NL, DI, DS)
    Af = np.ascontiguousarray(
        A.reshape(NL, NDC, 128, DS).transpose(0, 2, 1, 3), dtype=np.float32)

    def pcol(a):   # (NL, DI) -> (NL, 128, NDC), di = c*128 + p
        return np.ascontiguousarray(
            a.reshape(NL, NDC, 128).transpose(0, 2, 1), dtype=np.float32)

    gw = (ln_g * head_w).astype(np.float32)
    gwt = bf(gw.reshape(NF, 128).T)
    gsum = np.full((1, 1), gw.sum(), np.float32)
    cb = float((ln_b * head_w).sum() + head_b)
    cbrow = np.full((1, L), cb, np.float32)

    shared = {
        "emb": bf(emb), "win": win, "convd": bf(convd), "wx": wx,
        "wdt": wdt, "wout": wout, "convb": pcol(conv_b), "bdt": pcol(bdt),
        "Dp": pcol(Dp), "A": Af, "gw": gwt, "gsum": gsum, "cbrow": cbrow,
        "ident": bf(np.eye(128, dtype=np.float32)),
    }
    in_maps = []
    for bi in range(NCORES):
        xb = x[bi].astype(np.float32)
        rhs2 = np.stack([1.0 - xb, xb]).astype(BF_NP)   # (2, L)
        m = dict(shared)
        m["rhs2"] = rhs2
        in_maps.append(m)

    _last_in_maps = in_maps
    res = run_bass_kernel_spmd(nc, in_maps, list(range(NCORES)))
    out = np.stack([res.results[i]["logits"][0] for i in range(NCORES)])
    return out.astype(np.float32)
